# revision 12
# baseline (speedup 1.0000x reference)
"""Trainium2 Bass kernel for nn_Decouple (gnn_message_passing).

Per timestep l (L=8, one per NeuronCore):
  g = softmax(graph)          -> top-8 per row handled as threshold mask
  emb = graph @ fc_w.T + b    -> masked-softmax matmul replaces gather
  q/k/v = LN(emb2 @ w.T + b)  -> bn_stats layernorm, gains applied transposed
  scores = q k^T / sqrt(N)    -> kept transposed; A = exp(sT), B = 1/A
  cheb diffusion + projection -> z_V, z_I  [N, N] each

Sharding: L=8 timesteps across 8 cores, fully independent (no collectives).
"""

import math
import os
import sys

for _p in ("/opt/trn_rl_repo", "/root/.axon_site/_ro/trn_rl_repo"):
    if os.path.isdir(_p) and _p not in sys.path:
        sys.path.insert(0, _p)

import numpy as np

import concourse.bass as bass
from concourse import bacc, mybir
from concourse.masks import make_identity
from concourse.tile import TileContext

F32 = mybir.dt.float32
AF = mybir.ActivationFunctionType
OP = mybir.AluOpType

NCORES = 8
H = 64


def _chunks(n, c=128):
    out = []
    s = 0
    while s < n:
        out.append((s, min(c, n - s)))
        s += c
    return out


def build_nc(N):
    """One-core program; SPMD across 8 cores with different graph slices."""
    CH = _chunks(N, 128)   # row/col chunks of 128 (last may be short)
    NJ = _chunks(N, 512)   # 512-wide column chunks for matmul moving dim
    nch = len(CH)

    nc = bacc.Bacc()
    g_in = nc.declare_dram_parameter("graph", [N, N], F32, isOutput=False)
    fcwT = nc.declare_dram_parameter("fcwT", [N, 8], F32, isOutput=False)
    fcb = nc.declare_dram_parameter("fcb", [8, 1], F32, isOutput=False)
    qkvw = nc.declare_dram_parameter("qkvw", [9, 192], F32, isOutput=False)
    lng = nc.declare_dram_parameter("lng", [H, 3], F32, isOutput=False)
    lnb = nc.declare_dram_parameter("lnb", [H, 3], F32, isOutput=False)
    evn = nc.declare_dram_parameter("evn", [N, H], F32, isOutput=False)
    evt = nc.declare_dram_parameter("evt", [H, N], F32, isOutput=False)
    gw1 = nc.declare_dram_parameter("gw1", [128, N], F32, isOutput=False)
    gw2 = nc.declare_dram_parameter("gw2", [65, N], F32, isOutput=False)
    zv_o = nc.declare_dram_parameter("zv", [N, N], F32, isOutput=True)
    zi_o = nc.declare_dram_parameter("zi", [N, N], F32, isOutput=True)

    with TileContext(nc) as tc:
        with (
            tc.tile_pool(name="const", bufs=1) as cpool,
            tc.tile_pool(name="big", bufs=1) as bigp,
            tc.tile_pool(name="hold", bufs=1) as hold,
            tc.tile_pool(name="work", bufs=2) as work,
            tc.tile_pool(name="rowp", bufs=2) as rowp,
            tc.tile_pool(name="htp", bufs=3) as htp,
            tc.tile_pool(name="zput", bufs=2) as zput,
            tc.tile_pool(name="pst", bufs=2, space="PSUM") as pst,
            tc.tile_pool(name="psm", bufs=2, space="PSUM") as psm,
            tc.tile_pool(name="psx", bufs=2, space="PSUM") as psx,
            tc.tile_pool(name="psz", bufs=2, space="PSUM") as psz,
        ):
            # ---- constants / weights ----
            ident = cpool.tile([128, 128], F32)
            make_identity(nc, ident)
            eps_t = cpool.tile([128, 1], F32)
            nc.vector.memset(eps_t, 1e-5)
            fcw_t = cpool.tile([128, nch, 8], F32)
            nfull = (nch - 1) * 128
            nc.sync.dma_start(
                out=fcw_t[:, : nch - 1, :],
                in_=fcwT[0:nfull, :].rearrange("(c p) k -> p c k", p=128),
            )
            nc.sync.dma_start(out=fcw_t[: CH[-1][1], nch - 1, :], in_=fcwT[nfull:N, :])
            fcb_t = cpool.tile([8, 1], F32)
            nc.sync.dma_start(out=fcb_t, in_=fcb[:, :])
            qkvw_t = cpool.tile([9, 192], F32)
            nc.sync.dma_start(out=qkvw_t, in_=qkvw[:, :])
            lng_t = cpool.tile([H, 3], F32)
            nc.sync.dma_start(out=lng_t, in_=lng[:, :])
            lnb_t = cpool.tile([H, 3], F32)
            nc.sync.dma_start(out=lnb_t, in_=lnb[:, :])
            evn_t = cpool.tile([128, nch, H], F32)
            nc.sync.dma_start(
                out=evn_t[:, : nch - 1, :],
                in_=evn[0:nfull, :].rearrange("(c p) h -> p c h", p=128),
            )
            nc.sync.dma_start(out=evn_t[: CH[-1][1], nch - 1, :], in_=evn[nfull:N, :])
            evt_t = cpool.tile([H, N], F32)
            nc.sync.dma_start(out=evt_t, in_=evt[:, :])
            gw1_t = cpool.tile([128, N], F32)
            nc.sync.dma_start(out=gw1_t, in_=gw1[:, :])
            gw2_t = cpool.tile([65, N], F32)
            nc.sync.dma_start(out=gw2_t, in_=gw2[:, :])

            # ---- persistent mid-size tiles ----
            EmT = bigp.tile([128, nch * N], F32, tag="big")
            embT = rowp.tile([8, N], F32, tag="rowT")
            embn = hold.tile([128, nch * 8], F32)
            rcg = hold.tile([128, nch], F32)

            # ============ P1: graph streaming ============
            for i, (si, pi) in enumerate(CH):
                gnat = work.tile([128, N], F32, tag="gnat")
                nc.sync.dma_start(out=gnat[:pi, :], in_=g_in[si : si + pi, :])
                t8 = work.tile([128, 8], F32, tag="t8")
                nc.vector.max(t8[:pi, :], gnat[:pi, :])
                enat = work.tile([128, N], F32, tag="enat")
                rsg = work.tile([128, 1], F32, tag="rsg")
                nc.scalar.activation(
                    out=enat[:pi, :], in_=gnat[:pi, :], func=AF.Exp,
                    accum_out=rsg[:pi, :],
                )
                nc.vector.reciprocal(rcg[:pi, i : i + 1], rsg[:pi, :])
                nc.vector.scalar_tensor_tensor(
                    out=enat[:pi, :], in0=gnat[:pi, :], scalar=t8[:pi, 7:8],
                    in1=enat[:pi, :], op0=OP.is_ge, op1=OP.mult,
                )
                ps_e = psm.tile([128, 192], F32, tag="psmall")
                # transpose gnat + emask blocks; accumulate embT
                for g0 in range(0, nch, 4):
                    mcs = [mc for mc in range(g0, min(g0 + 4, nch))]
                    psT = pst.tile([128, 512], F32, tag="pst")
                    for c, mc in enumerate(mcs):
                        sm, pm = CH[mc]
                        nc.tensor.transpose(
                            psT[:pm, c * pi : c * pi + pi],
                            gnat[:pi, sm : sm + pm],
                            ident[:pi, :pi],
                        )
                    gtb = work.tile([128, 512], F32, tag="gtb")
                    ncols = len(mcs) * pi
                    pmax = CH[mcs[0]][1]
                    nc.scalar.copy(gtb[:pmax, :ncols], psT[:pmax, :ncols])
                    for c, mc in enumerate(mcs):
                        sm, pm = CH[mc]
                        nc.tensor.matmul(
                            ps_e[:8, :pi],
                            fcw_t[:pm, mc, :],
                            gtb[:pm, c * pi : c * pi + pi],
                            start=(mc == 0),
                            stop=(mc == nch - 1),
                        )
                    psM = pst.tile([128, 512], F32, tag="pst")
                    for c, mc in enumerate(mcs):
                        sm, pm = CH[mc]
                        nc.tensor.transpose(
                            psM[:pm, c * pi : c * pi + pi],
                            enat[:pi, sm : sm + pm],
                            ident[:pi, :pi],
                        )
                    off = nch * si + mcs[0] * pi
                    nc.scalar.copy(EmT[:pmax, off : off + ncols], psM[:pmax, :ncols])
                nc.scalar.activation(
                    out=embT[:, si : si + pi], in_=ps_e[:8, :pi], func=AF.Identity,
                    bias=fcb_t,
                )

            # ============ P2: neighbor aggregation (emb2) ============
            for i, (si, pi) in enumerate(CH):
                psn = pst.tile([128, 512], F32, tag="pst")
                nc.tensor.transpose(psn[:pi, :8], embT[:, si : si + pi], ident[:8, :8])
                nc.scalar.copy(embn[:pi, i * 8 : i * 8 + 8], psn[:pi, :8])
            e2uT = rowp.tile([8, N], F32, tag="rowT")
            for i, (si, pi) in enumerate(CH):
                ps_e = psm.tile([128, 192], F32, tag="psmall")
                for mc in range(nch):
                    sm, pm = CH[mc]
                    off = nch * si + mc * pi
                    nc.tensor.matmul(
                        ps_e[:8, :pi],
                        embn[:pm, mc * 8 : mc * 8 + 8],
                        EmT[:pm, off : off + pi],
                        start=(mc == 0),
                        stop=(mc == nch - 1),
                    )
                nc.scalar.copy(e2uT[:, si : si + pi], ps_e[:8, :pi])
            e2n = hold.tile([128, nch * 8], F32)
            for i, (si, pi) in enumerate(CH):
                psn = pst.tile([128, 512], F32, tag="pst")
                nc.tensor.transpose(psn[:pi, :8], e2uT[:, si : si + pi], ident[:8, :8])
                nc.vector.scalar_tensor_tensor(
                    out=e2n[:pi, i * 8 : i * 8 + 8], in0=psn[:pi, :8],
                    scalar=rcg[:pi, i : i + 1], in1=embn[:pi, i * 8 : i * 8 + 8],
                    op0=OP.mult, op1=OP.add,
                )
            e2T = rowp.tile([9, N], F32, tag="rowT")
            nc.vector.memset(e2T, 1.0)
            for i, (si, pi) in enumerate(CH):
                psn = pst.tile([128, 512], F32, tag="pst")
                nc.tensor.transpose(
                    psn[:8, :pi], e2n[:pi, i * 8 : i * 8 + 8], ident[:pi, :pi]
                )
                nc.scalar.copy(e2T[:8, si : si + pi], psn[:8, :pi])

            # ============ P3: q/k/v + layernorm ============
            qT = htp.tile([128, N], F32, tag="hT", name="qT")[:H]
            kT = htp.tile([128, N], F32, tag="hT", name="kT")[:H]
            vT = htp.tile([128, N], F32, tag="hT", name="vT")[:H]
            for i, (si, pi) in enumerate(CH):
                ps_q = psm.tile([128, 192], F32, tag="psmall")
                nc.tensor.matmul(ps_q[:pi, :], e2T[:, si : si + pi], qkvw_t)
                qkvn = work.tile([128, 192], F32, tag="qkvn")
                for s in range(3):
                    sl = ps_q[:pi, s * 64 : s * 64 + 64]
                    st6 = work.tile([128, 6], F32, tag="st6")
                    nc.vector.bn_stats(out=st6[:pi, :], in_=sl)
                    mv = work.tile([128, 2], F32, tag="mv")
                    nc.vector.bn_aggr(out=mv[:pi, :], in_=st6[:pi, :])
                    sd = work.tile([128, 1], F32, tag="sd")
                    nc.scalar.activation(
                        out=sd[:pi, :], in_=mv[:pi, 1:2], func=AF.Sqrt,
                        bias=eps_t[:pi, :],
                    )
                    rstd = work.tile([128, 1], F32, tag="rstd")
                    nc.vector.reciprocal(rstd[:pi, :], sd[:pi, :])
                    nc.vector.tensor_scalar(
                        out=qkvn[:pi, s * 64 : s * 64 + 64], in0=sl,
                        scalar1=mv[:pi, 0:1], scalar2=rstd[:pi, :],
                        op0=OP.subtract, op1=OP.mult,
                    )
                for s, dst in enumerate((qT, kT, vT)):
                    psn = pst.tile([128, 512], F32, tag="pst")
                    nc.tensor.transpose(
                        psn[:64, :pi], qkvn[:pi, s * 64 : s * 64 + 64], ident[:pi, :pi]
                    )
                    nc.vector.tensor_scalar(
                        out=dst[:, si : si + pi], in0=psn[:64, :pi],
                        scalar1=lng_t[:, s : s + 1], scalar2=lnb_t[:, s : s + 1],
                        op0=OP.mult, op1=OP.add,
                    )
            vna = hold.tile([128, nch * 65], F32)
            vVn = hold.tile([128, nch * 65], F32)
            nc.vector.memset(vna, 1.0)
            nc.vector.memset(vVn, 1.0)
            for i, (si, pi) in enumerate(CH):
                psn = pst.tile([128, 512], F32, tag="pst")
                nc.tensor.transpose(psn[:pi, :64], vT[:, si : si + pi], ident[:64, :64])
                nc.scalar.copy(vna[:pi, i * 65 : i * 65 + 64], psn[:pi, :64])
                nc.vector.tensor_tensor(
                    out=vVn[:pi, i * 65 : i * 65 + 64], in0=psn[:pi, :64],
                    in1=evn_t[:pi, i, :], op=OP.mult,
                )

            # ============ P4: scores -> A = exp(sT) ============
            A = bigp.tile([128, nch * N], F32, tag="big")
            for mc in range(nch):
                sm, pm = CH[mc]
                for jo, js in NJ:
                    ps_s = psz.tile([128, 512], F32, tag="psz")
                    nc.tensor.matmul(
                        ps_s[:pm, :js], kT[:, sm : sm + pm], qT[:, jo : jo + js]
                    )
                    nc.scalar.activation(
                        out=A[:pm, mc * N + jo : mc * N + jo + js],
                        in_=ps_s[:pm, :js], func=AF.Exp,
                    )

            # ============ P5: chebyshev + projection, both sides ============
            xsT = htp.tile([128, N], F32, tag="hT")
            x2T = htp.tile([128, N], F32, tag="hT", name="x2T")[:65]
            for side in range(2):
                val = vna if side == 0 else vVn
                zout = zi_o if side == 0 else zv_o
                if side == 1:
                    for mc in range(nch):
                        sm, pm = CH[mc]
                        blk = A[:pm, mc * N : mc * N + N]
                        nc.vector.reciprocal(blk, blk)
                    nc.vector.tensor_tensor(
                        out=xsT[0:64, :], in0=vT, in1=evt_t, op=OP.mult
                    )
                else:
                    nc.scalar.copy(xsT[0:64, :], vT)
                nc.vector.memset(x2T[64:65, :], 1.0)
                # x1u = [val | 1]^T thru A
                x1u = work.tile([65, N], F32, tag="x1u")
                for jo, js in NJ:
                    ps_x = psx.tile([65, 512], F32, tag="psx")
                    for mc in range(nch):
                        sm, pm = CH[mc]
                        nc.tensor.matmul(
                            ps_x[:, :js],
                            val[:pm, mc * 65 : mc * 65 + 65],
                            A[:pm, mc * N + jo : mc * N + jo + js],
                            start=(mc == 0),
                            stop=(mc == nch - 1),
                        )
                    nc.scalar.copy(x1u[:, jo : jo + js], ps_x[:, :js])
                rc = work.tile([128, nch], F32, tag="rc")
                nc.vector.memset(rc, 1.0)
                x1n = work.tile([128, nch * 64], F32, tag="x1n")
                for i, (si, pi) in enumerate(CH):
                    psn = pst.tile([128, 512], F32, tag="pst")
                    nc.tensor.transpose(
                        psn[:pi, :65], x1u[:, si : si + pi], ident[:65, :65]
                    )
                    nc.vector.reciprocal(rc[:pi, i : i + 1], psn[:pi, 64:65])
                    nc.vector.tensor_scalar(
                        out=x1n[:pi, i * 64 : i * 64 + 64], in0=psn[:pi, :64],
                        scalar1=rc[:pi, i : i + 1], scalar2=None, op0=OP.mult,
                    )
                for i, (si, pi) in enumerate(CH):
                    psn = pst.tile([128, 512], F32, tag="pst")
                    nc.tensor.transpose(
                        psn[:64, :pi], x1n[:pi, i * 64 : i * 64 + 64], ident[:pi, :pi]
                    )
                    nc.scalar.copy(xsT[64:128, si : si + pi], psn[:64, :pi])
                x2u = work.tile([64, N], F32, tag="x1u")
                for jo, js in NJ:
                    ps_x2 = psx.tile([65, 512], F32, tag="psx")
                    for mc in range(nch):
                        sm, pm = CH[mc]
                        nc.tensor.matmul(
                            ps_x2[:64, :js],
                            x1n[:pm, mc * 64 : mc * 64 + 64],
                            A[:pm, mc * N + jo : mc * N + jo + js],
                            start=(mc == 0),
                            stop=(mc == nch - 1),
                        )
                    nc.scalar.copy(x2u[:, jo : jo + js], ps_x2[:64, :js])
                rc2 = work.tile([128, nch], F32, tag="rc2")
                nc.vector.tensor_scalar(
                    out=rc2, in0=rc, scalar1=2.0, scalar2=None, op0=OP.mult
                )
                for i, (si, pi) in enumerate(CH):
                    psn = pst.tile([128, 512], F32, tag="pst")
                    nc.tensor.transpose(
                        psn[:pi, :64], x2u[:, si : si + pi], ident[:64, :64]
                    )
                    x2b = work.tile([128, 64], F32, tag="x2b")
                    nc.vector.scalar_tensor_tensor(
                        out=x2b[:pi, :], in0=psn[:pi, :64],
                        scalar=rc2[:pi, i : i + 1],
                        in1=val[:pi, i * 65 : i * 65 + 64],
                        op0=OP.mult, op1=OP.subtract,
                    )
                    psn2 = pst.tile([128, 512], F32, tag="pst")
                    nc.tensor.transpose(psn2[:64, :pi], x2b[:pi, :], ident[:pi, :pi])
                    nc.scalar.copy(x2T[0:64, si : si + pi], psn2[:64, :pi])
                # z projection
                for i, (si, pi) in enumerate(CH):
                    zsb = zput.tile([128, N], F32, tag="zsb")
                    for jc, (jo, js) in enumerate(NJ):
                        ps_z = psz.tile([128, 512], F32, tag="psz")
                        nc.tensor.matmul(
                            ps_z[:pi, :js], xsT[:, si : si + pi],
                            gw1_t[:, jo : jo + js], start=True, stop=False,
                        )
                        nc.tensor.matmul(
                            ps_z[:pi, :js], x2T[:, si : si + pi],
                            gw2_t[:, jo : jo + js], start=False, stop=True,
                        )
                        if jc % 2 == 0:
                            nc.scalar.copy(zsb[:pi, jo : jo + js], ps_z[:pi, :js])
                        else:
                            nc.vector.tensor_copy(zsb[:pi, jo : jo + js], ps_z[:pi, :js])
                    nc.sync.dma_start(out=zout[si : si + pi, :], in_=zsb[:pi, :])
    nc.compile()
    return nc


def host_prep(N, fc_w, fc_b, q_w, q_b, k_w, k_b, v_w, v_b,
              ln1_g, ln1_b, ln2_g, ln2_b, ln3_g, ln3_b, E1, E2, g_w, g_b):
    f32 = lambda x: np.ascontiguousarray(x, dtype=np.float32)
    s = 1.0 / math.sqrt(N)
    w = {}
    w["fcwT"] = f32(fc_w.T)
    w["fcb"] = f32(np.asarray(fc_b).reshape(8, 1))
    top = np.concatenate([q_w.T, k_w.T, v_w.T], axis=1)
    bot = np.concatenate([q_b, k_b, v_b]).reshape(1, 192)
    w["qkvw"] = f32(np.concatenate([top, bot], axis=0))
    w["lng"] = f32(np.stack([ln1_g * s, ln2_g, ln3_g], axis=1))
    w["lnb"] = f32(np.stack([ln1_b * s, ln2_b, ln3_b], axis=1))
    e = np.asarray(E1, np.float64) @ np.asarray(E2, np.float64)
    e = e - e.max(axis=-1, keepdims=True)
    ev = np.exp(e)
    ev = ev / ev.sum(axis=-1, keepdims=True)
    w["evn"] = f32(ev)
    w["evt"] = f32(ev.T)
    gwr = np.asarray(g_w).reshape(N, 64, 3).transpose(0, 2, 1).reshape(N, 192)
    gaug = np.concatenate([gwr.T, np.asarray(g_b).reshape(1, N)], axis=0)
    w["gw1"] = f32(gaug[0:128])
    w["gw2"] = f32(gaug[128:193])
    return w


_NC_CACHE = {}


def kernel(graph_list, **weights):
    from concourse.bass_utils import run_bass_kernel_spmd

    graph_list = np.ascontiguousarray(graph_list, dtype=np.float32)
    L, N, _ = graph_list.shape
    assert L == NCORES
    w = host_prep(N, **weights)
    if N not in _NC_CACHE:
        _NC_CACHE[N] = build_nc(N)
    nc = _NC_CACHE[N]
    in_maps = [dict(graph=graph_list[c], **w) for c in range(NCORES)]
    res = run_bass_kernel_spmd(nc, in_maps, list(range(NCORES)))
    z_v = np.stack([res.results[c]["zv"] for c in range(NCORES)])
    z_i = np.stack([res.results[c]["zi"] for c in range(NCORES)])
    return (z_v, z_i)
